# revision 1
# baseline (speedup 1.0000x reference)
"""GCN message-passing kernel for 8 TRN2 NeuronCores (bf16 datapath).

Reference computation (per (b, c) pair):
    e1  = x @ W1^T + b1          [N, H]
    e2  = x @ W2^T + b2          [N, H]
    adj = relu(e1 @ e2^T)        [N, N]
    h   = adj @ x                [N, F]
    out = h @ W3^T + b3          [N, O]

Sharding: the 32 (b, c) pairs are split 4-per-core across 8 cores; weights
are replicated. Each core runs an identical Bass program fully fused in
SBUF/PSUM (the N x N adjacency never touches HBM).

v2 design vs the fp32r baseline:
- All matmul operands are bf16 (PSUM accumulation stays fp32), which
  enables the fast weight-load path (FWL is fp32-excluded) and halves
  SBUF streaming traffic. Tolerance is 2e-2; bf16 lands ~1e-3.
- x arrives pre-cast (xb) and pre-transposed (xt) from the host, so the
  on-device PE transposes and fp32->bf16 casts of the baseline are gone.
- Weights arrive pre-transposed/augmented: wa1/wa2 = [W^T; b] duplicated
  along M so the e1^T/e2^T matmuls fill all 128 output partitions -- the
  lower 64 partitions hold a second copy that feeds PE row-tile B.
- Adjacency matmuls are K=64: two run CONCURRENTLY in the PE array as
  2x row tiles (partitions 0-63 / 64-127), halving adjacency PE time.
- Propagate (h) matmuls are M=64: two run concurrently as 2x col tiles
  writing disjoint partition halves of a shared PSUM bank.
- Output projection is computed transposed (out^T = W3 @ h^T + b3) as two
  diagonal-quadrant PE tiles; b3 is folded into the PSUM->SBUF copy via
  scalar activation Identity bias. The host undoes the tile layout.
"""

import sys

for _p in ("/opt/trn_rl_repo",):
    if _p not in sys.path:
        sys.path.insert(0, _p)

import numpy as np

import concourse.bass as bass
import concourse.tile as tile
from concourse import bacc, mybir
from concourse.bass import ts

B, C, N, F = 4, 8, 2048, 64
H = 64
O = 64
NCORES = 8
PAIRS = (B * C) // NCORES  # 4 (b,c) pairs per core
P = 128                    # SBUF partitions
TBLK = N // P              # 16 row-blocks per pair
CH = 512                   # moving-operand chunk (one PSUM bank of fp32)
NCH = N // CH              # 4 chunks per row
F32 = mybir.dt.float32
BF16 = mybir.dt.bfloat16

AF = mybir.ActivationFunctionType


class _EngineAlternator:
    """Round-robin PSUM->SBUF copy/relu work across Scalar and Vector."""

    def __init__(self, nc):
        self.nc = nc
        self.i = 0

    def copy(self, out, in_):
        self.i += 1
        if self.i % 2:
            self.nc.scalar.copy(out, in_)
        else:
            self.nc.vector.tensor_copy(out, in_)

    def relu(self, out, in_):
        self.i += 1
        if self.i % 2:
            self.nc.scalar.activation(out, in_, AF.Relu)
        else:
            self.nc.vector.tensor_scalar_max(out, in_, 0.0)


def _emit(tc, xb_d, xt_d, wa_d, w3t_d, out_d, reps=1):
    nc = tc.nc
    eng = _EngineAlternator(nc)

    import contextlib

    with contextlib.ExitStack() as ctx:
        consts = ctx.enter_context(tc.tile_pool(name="consts", bufs=1))
        xpool = ctx.enter_context(tc.tile_pool(name="xp", bufs=2))
        xtapool = ctx.enter_context(tc.tile_pool(name="xta", bufs=2))
        epool = ctx.enter_context(tc.tile_pool(name="ep", bufs=2))
        adjpool = ctx.enter_context(tc.tile_pool(name="adj", bufs=8))
        htpool = ctx.enter_context(tc.tile_pool(name="ht", bufs=2))
        opool = ctx.enter_context(tc.tile_pool(name="op", bufs=2))
        # 6-deep shared ring for adjacency + prologue/epilogue psum (tag
        # "pa"), 2 banks for the h accumulators: 8 banks total.
        ps_adj = ctx.enter_context(tc.tile_pool(name="psa", bufs=6, space="PSUM"))
        ps_h = ctx.enter_context(tc.tile_pool(name="psh", bufs=2, space="PSUM"))
        ps_m = ps_adj

        # Replicated constants (tiny): wa1/wa2 [F+1, 128] bf16, w3t2 [128, O]
        # bf16, b3t [128, 1] fp32.
        was = []
        for k in range(2):
            wa = consts.tile([F + 1, P], BF16, tag=f"wa{k}")
            nc.sync.dma_start(wa[:], wa_d[k][:])
            was.append(wa)
        w1a, w2a = was
        w3t2 = consts.tile([P, O], BF16, tag="w3t2")
        nc.sync.dma_start(w3t2[:], w3t_d[:])

        def prep(p):
            """Pair prologue: xb load, xT load (+ ones row), e1T/e2T matmuls.
            Emitted mid-way through the previous pair's main loop so the PE
            queue never drains at a pair boundary."""
            # xb is host-permuted to [P, TBLK*F]: partition q block t holds
            # x row t*128+q, so xb_sb[:, ts(mb, F)] is natural rows
            # mb*128..mb*128+127 -- matching the adjacency block layout.
            xb_sb = xpool.tile([P, TBLK * F], BF16, tag="xb", name=f"xb{p}")
            nc.sync.dma_start(xb_sb[:], xb_d[p][:])
            # xt arrives with the ones row appended (row F), so K=65 biases
            # come in the same DMA
            xta = xtapool.tile([F + 1, N], BF16, tag="xta", name=f"xta{p}")
            nc.sync.dma_start(xta[:], xt_d[p][:])

            # e1T / e2T = Wa^T @ xTa (biases via K=65); output partitions
            # 0-63 and 64-127 both hold e^T (wa is M-duplicated) so PE row
            # tiles A/B each have a private partition-range copy.
            ets = []
            for wa, tag in ((w1a, "e1t"), (w2a, "e2t")):
                et = epool.tile([P, N], BF16, tag=tag, name=f"{tag}{p}")
                for c in range(NCH):
                    pe_ = ps_m.tile([P, CH], F32, tag="pa", name=f"pe{tag}{p}_{c}")
                    nc.tensor.matmul(
                        pe_[:], wa[:], xta[:, ts(c, CH)], start=True, stop=True
                    )
                    eng.copy(et[:, ts(c, CH)], pe_[:])
                ets.append(et)
            return xb_sb, ets[0], ets[1]

        def main(p, st, tail_emit):
            """Main fused loop for pair p. Interleaves the deferred tail of
            pair p-1 (early) and the prologue of pair p+1 (late)."""
            xb_sb, e1t, e2t = st
            ph = [
                ps_h.tile([P, CH], F32, tag="ph", name=f"ph{p}_{k}")
                for k in range(2)
            ]
            next_st = None

            def emit_adj(mb):
                # Two concurrent PE row tiles: even chunks on partitions 0-63
                # (tile (0,0)), odd on 64-127 (tile (64,0)). Emission order
                # 0,2,1,3 keeps each tile's stationary back-to-back so the
                # redundant LDWEIGHTS is skipped, and tile B's load overlaps
                # tile A's matmuls (different row groups).
                pas = [None] * NCH
                for c in range(NCH):
                    lo, hi = (0, H) if c % 2 == 0 else (H, P)
                    pa = ps_adj.tile([P, CH], F32, tag="pa",
                                     name=f"pa{p}_{mb}_{c}")
                    nc.tensor.matmul(
                        pa[:], e2t[lo:hi, ts(mb, P)], e1t[lo:hi, ts(c, CH)],
                        start=True, stop=True,
                    )
                    pas[c] = pa
                return pas

            # one-block lookahead: adj(mb+1) is emitted (and thus scheduled
            # on the PE) BEFORE h(mb), which has to wait for relu(mb) -- so
            # the PE chews adjacency blocks while the relu engines drain.
            pas = emit_adj(0)
            for mb in range(TBLK):
                # Fixed chunk->engine assignment with per-engine tile tags:
                # a shared tag ring serializes ACT/DVE against each other
                # (HW-benched 726 -> 320 ns/op with separate tags).
                asbs = []
                for c in range(NCH):
                    asb = adjpool.tile([P, CH], BF16, tag=f"asb{c % 2}",
                                       name=f"asb{p}_{mb}_{c}")
                    if c % 2 == 0:
                        nc.scalar.activation(asb[:], pas[c][:], AF.Relu)
                    else:
                        nc.vector.tensor_scalar_max(asb[:], pas[c][:], 0.0)
                    asbs.append(asb)
                if mb + 1 < TBLK:
                    pas = emit_adj(mb + 1)
                # Two concurrent PE col tiles per ph bank: even chunks ->
                # partitions 0-63 (tile (0,0)), odd -> 64-127 (tile (0,64)).
                for c in range(NCH):
                    k, lo = c // 2, (0 if c % 2 == 0 else H)
                    nc.tensor.matmul(
                        ph[k][lo : lo + H, :], xb_sb[:, ts(mb, F)], asbs[c][:],
                        start=(mb == 0), stop=(mb == TBLK - 1),
                        skip_group_check=True,
                    )
                if mb == 3 and tail_emit is not None:
                    tail_emit()
                    tail_emit = None
                if mb == 8 and p + 1 < PAIRS:
                    next_st = prep(p + 1)

            # h^T -> SBUF right away (frees the ph PSUM banks; DVE/ACT work
            # that overlaps the next pair's PE stream)
            hta = htpool.tile([P, 2 * CH], BF16, tag="hta", name=f"hta{p}")
            for k in range(2):
                eng.copy(hta[:, ts(k, CH)], ph[k][:])

            def tail():
                # out^T = W3 @ h^T, two diagonal PE quadrant tiles per
                # n-chunk pair: (0,0) for partitions 0-63, (64,64) for
                # 64-127. b3 is added on the host.
                out_sb = opool.tile([P, 2 * CH], F32, tag="out_sb",
                                    name=f"out_sb{p}")
                for k in range(2):
                    po = ps_m.tile([P, CH], F32, tag="pa", name=f"po{p}_{k}")
                    nc.tensor.matmul(
                        po[0:H, :], w3t2[0:H, :], hta[0:H, ts(k, CH)],
                        start=True, stop=True, skip_group_check=True,
                    )
                    nc.tensor.matmul(
                        po[H:P, :], w3t2[H:P, :], hta[H:P, ts(k, CH)],
                        start=True, stop=True, skip_group_check=True,
                    )
                    eng.copy(out_sb[:, ts(k, CH)], po[:])
                nc.sync.dma_start(out_d[p][:], out_sb[:])

            return next_st, tail

        def body():
            st = prep(0)
            tail = None
            for p in range(PAIRS):
                st, tail = main(p, st, tail)
            tail()

        if reps == 1:
            body()
        else:
            with tc.For_i(0, reps, 1):
                body()


def build_program(reps=1):
    nc = bacc.Bacc("TRN2", target_bir_lowering=False, debug=False)
    xb_d = nc.dram_tensor(
        "xb", [PAIRS, P, TBLK * F], BF16, kind="ExternalInput"
    ).ap()
    xt_d = nc.dram_tensor(
        "xt", [PAIRS, F + 1, N], BF16, kind="ExternalInput"
    ).ap()
    wa_d = [
        nc.dram_tensor(f"wa{k}", [F + 1, P], BF16, kind="ExternalInput").ap()
        for k in (1, 2)
    ]
    w3t_d = nc.dram_tensor("w3t2", [P, O], BF16, kind="ExternalInput").ap()
    out_d = nc.dram_tensor(
        "out", [PAIRS, P, 2 * CH], F32, kind="ExternalOutput"
    ).ap()
    with tile.TileContext(nc) as tc:
        _emit(tc, xb_d, xt_d, wa_d, w3t_d, out_d, reps=reps)
    nc.compile()
    return nc


def make_in_maps(x, W1, b1, W2, b2, W3, b3):
    bf16 = mybir.dt.np(BF16)
    xs = np.asarray(x, np.float32).reshape(B * C, N, F)
    # partition q, block t <- x row t*128+q (see prep())
    xb = np.ascontiguousarray(
        xs.reshape(-1, TBLK, P, F).transpose(0, 2, 1, 3)
        .reshape(-1, P, TBLK * F).astype(bf16)
    )
    xtc = xs.transpose(0, 2, 1).astype(bf16)          # [pairs, F, N]
    ones = np.ones((xtc.shape[0], 1, N), bf16)
    xt = np.ascontiguousarray(np.concatenate([xtc, ones], 1))

    def aug(Wk, bk):
        a = np.concatenate(
            [np.asarray(Wk, np.float32).T, np.asarray(bk, np.float32)[None]], 0
        )
        return np.ascontiguousarray(np.concatenate([a, a], 1).astype(bf16))

    w3t = np.asarray(W3, np.float32).T.astype(bf16)
    const = {
        "wa1": aug(W1, b1),
        "wa2": aug(W2, b2),
        "w3t2": np.ascontiguousarray(np.concatenate([w3t, w3t], 0)),
    }
    return [
        {
            "xb": np.ascontiguousarray(xb[i * PAIRS : (i + 1) * PAIRS]),
            "xt": np.ascontiguousarray(xt[i * PAIRS : (i + 1) * PAIRS]),
            **const,
        }
        for i in range(NCORES)
    ]


def unpack_out(raw, b3):
    """[PAIRS, 128, 1024] raw tile layout -> [PAIRS, N, O] (+ b3).

    raw[ph*64+o, cc*512+j] = out[cc*1024 + ph*512 + j, o]
    """
    r = raw.reshape(-1, 2, O, 2, CH)          # [pairs, ph, o, cc, j]
    out = r.transpose(0, 3, 1, 4, 2).reshape(-1, N, O)
    return out + np.asarray(b3, np.float32)


_NC_CACHE = {}


def kernel(x, W1, b1, W2, b2, W3, b3):
    from concourse.bass_utils import run_bass_kernel_spmd

    if "nc" not in _NC_CACHE:
        _NC_CACHE["nc"] = build_program()
    nc = _NC_CACHE["nc"]
    in_maps = make_in_maps(x, W1, b1, W2, b2, W3, b3)
    res = run_bass_kernel_spmd(nc, in_maps, list(range(NCORES))).results
    out = np.concatenate(
        [unpack_out(res[i]["out"], b3) for i in range(NCORES)], axis=0
    )
    return out.reshape(B, C, N, O)

